# revision 1
# baseline (speedup 1.0000x reference)
"""Trainium2 Bass kernel for the depth-dependent camera rendering problem.

Strategy
--------
Host (numpy, float64): PSF synthesis (phase -> Hankel einsum -> radial
interp -> quadrant mirror -> fftshift -> normalize) and rfft2 of the PSF
(tiny: ~1% of FLOPs), plus input sharding.

Device (Bass/Tile, 8 NeuronCores, SPMD): 6 cores each own one (b, c)
chain.  Per core, a backward depth loop d = 15..0 fuses:
  layered_d = (idx == d); vol_d = layered_d * img/scale
  forward 2D DFT of both planes (matmul-only, no transposes)
  freq suffix-cumsum  Fcum += Flay
  3 complex products with Fpsf[c, d] (pre-normalized by 1/N^2)
  3 inverse 2D DFTs
  ba = alpha/(cum+eps); bv = vol/(cum+eps)
  A = bv + (1 - ba) * A          (back-to-front over-compositing)
Final A = captimg[b, c] / scale.

2D DFT via chained matmuls (out = lhsT.T @ rhs contracts the partition
axis and swaps the other two), so the plane layout ping-pongs and no
transposes are ever needed:
  [H, W] --c1--> [W, hf(384)] --c2--> [hf, wf(193)]   (forward, rfft2 conv)
  [hf, wf] --iA--> [wf, H] --iB--> [H, W]             (inverse)
"""

import os
import time

import numpy as np

import concourse.bass as bass
import concourse.tile as tile
from concourse import bacc, mybir
from concourse.bass_utils import run_bass_kernel_spmd

dt = mybir.dt
Alu = mybir.AluOpType

# ---- problem constants (hardcoded; kernel.py must be self-contained) ----
N = 384            # image H = W
HF = N             # full spectrum bins along H
WF = N // 2 + 1    # rfft bins along W = 193
D = 16             # depth planes
B, C = 2, 3
EPS = 1e-3
NCORES = 8
WAVELENGTHS = np.array([632e-9, 550e-9, 450e-9])
FOCAL_LENGTH = 50e-3
FOCAL_DEPTH = 1.7
SENSOR_DIST = 1.0 / (1.0 / FOCAL_LENGTH - 1.0 / FOCAL_DEPTH)

MM_DT = dt.float32r   # matmul operand mode (full-rate); set dt.float32 for precision


# =====================================================================
# Host-side DFT tables
# =====================================================================
def _make_tables():
    k = np.arange(N, dtype=np.float64)
    th = 2.0 * np.pi * np.outer(k, k) / N     # [N, N]
    co = np.cos(th)
    sn = np.sin(th)
    c1 = np.concatenate([co, -sn], axis=1)                      # [N, 2N]
    c2a = np.concatenate([co[:, :WF], -sn[:, :WF]], axis=1)     # [N, 2*WF]
    c2b = np.concatenate([sn[:, :WF], co[:, :WF]], axis=1)      # [N, 2*WF]
    b = np.full(WF, 2.0)
    b[0] = 1.0
    b[WF - 1] = 1.0
    ibr = b[:, None] * co[:WF, :]                               # [WF, N]
    ibi = -b[:, None] * sn[:WF, :]                              # [WF, N]
    ib = np.stack([ibr, ibi], axis=0)                           # [2, WF, N]
    return (c1.astype(np.float32), sn.astype(np.float32),
            c2a.astype(np.float32), c2b.astype(np.float32),
            ib.astype(np.float32))


def _fwd_np(x, c1, c2a, c2b):
    """Numpy mirror of the device forward DFT (for validation)."""
    x = x.astype(np.float32)
    y1 = x.T @ c1                                   # [W, 2N]: [Y1r | Y1i]
    y1r, y1i = y1[:, :N], y1[:, N:]
    # pass2: lhsT = y1 (contract W): out[hf, :] = sum_w y1[w, hf] * c2[w, :]
    z = y1r.T @ c2a + y1i.T @ c2b                   # [HF, 2*WF]
    return z


def _inv_np(z, c1, sn, ib):
    """Numpy mirror of the device inverse DFT (for validation)."""
    zr, zi = z[:, :WF], z[:, WF:]
    co = c1[:, :N]
    nsn = c1[:, N:]
    # stepA: P[wf, h]: lhsT = z chunks; Pr = Zr.T@co + Zi.T@(-sn); Pi = Zr.T@sn + Zi.T@co
    pr = zr.T @ co + zi.T @ nsn                     # [WF, N]
    pi = zr.T @ sn + zi.T @ co                      # [WF, N]
    # stepB: y[h, w] = sum_wf pr[wf, h]*ibr[wf, w] + pi[wf, h]*ibi[wf, w]
    y = pr.T @ ib[0] + pi.T @ ib[1]                 # [N, N]
    return y


# =====================================================================
# Device program
# =====================================================================
def build_program(occlusion: bool, n_depth: int = D):
    nc = bacc.Bacc(None, target_bir_lowering=False, debug=False)
    f32 = dt.float32

    img_d = nc.declare_dram_parameter("img", [N, N], f32, isOutput=False)
    idx_d = nc.declare_dram_parameter("idx", [N, N], f32, isOutput=False)
    pf_d = nc.declare_dram_parameter("fpsf", [D, 2, HF, WF], f32, isOutput=False)
    c1_d = nc.declare_dram_parameter("c1", [N, 2 * N], f32, isOutput=False)
    si_d = nc.declare_dram_parameter("si", [N, N], f32, isOutput=False)
    c2a_d = nc.declare_dram_parameter("c2a", [N, 2 * WF], f32, isOutput=False)
    c2b_d = nc.declare_dram_parameter("c2b", [N, 2 * WF], f32, isOutput=False)
    ib_d = nc.declare_dram_parameter("ib", [2, WF, N], f32, isOutput=False)
    out_d = nc.declare_dram_parameter("out", [N, N], f32, isOutput=True)

    PCH = [(0, 128), (128, 256), (256, 384)]          # partition chunks of 384
    WCH = [(0, 97), (97, WF)]                          # partition chunks of 193

    with tile.TileContext(nc) as tc:
        with (
            tc.tile_pool(name="const", bufs=1) as cp,
            tc.tile_pool(name="pers", bufs=1) as pp,
            tc.tile_pool(name="work", bufs=2) as wp,
            tc.tile_pool(name="spec", bufs=2) as sp,
            tc.tile_pool(name="y1p", bufs=2) as y1p,
            tc.tile_pool(name="pbp", bufs=2) as pbp,
            tc.tile_pool(name="pfp", bufs=2) as pfp,
            tc.tile_pool(name="psy1r", bufs=1, space="PSUM") as ps_y1r,
            tc.tile_pool(name="psy1i", bufs=1, space="PSUM") as ps_y1i,
            tc.tile_pool(name="psz", bufs=1, space="PSUM") as ps_z,
            tc.tile_pool(name="pspr", bufs=2, space="PSUM") as ps_pr,
            tc.tile_pool(name="pspi", bufs=2, space="PSUM") as ps_pi,
            tc.tile_pool(name="psy", bufs=1, space="PSUM") as ps_y,
        ):
            # ---- load constants ----
            def load3(dram, cols, tag, dtype=f32):
                ts = []
                for ci, (lo, hi) in enumerate(PCH):
                    t = cp.tile([128, cols], dtype, name=f"{tag}{ci}", tag=f"{tag}{ci}")
                    eng = nc.gpsimd if dtype != f32 else nc.sync
                    eng.dma_start(t[:], dram[lo:hi, :])
                    ts.append(t)
                return ts

            c1t = load3(c1_d, 2 * N, "c1", MM_DT)     # [cos | -sin] over [h, k]
            sit = load3(si_d, N, "si", MM_DT)         # sin
            c2at = load3(c2a_d, 2 * WF, "c2a", MM_DT)
            c2bt = load3(c2b_d, 2 * WF, "c2b", MM_DT)
            ibt = []                            # ib chunks: [2][wf-chunk]
            for comp in range(2):
                row = []
                for ci, (lo, hi) in enumerate(WCH):
                    t = cp.tile([hi - lo, N], MM_DT, name=f"ib{comp}{ci}", tag=f"ib{comp}{ci}")
                    nc.gpsimd.dma_start(t[:], ib_d[comp, lo:hi, :])
                    row.append(t)
                ibt.append(row)

            imgt = load3(img_d, N, "img")
            idxt = load3(idx_d, N, "idx")

            # persistent accumulators
            acct = [pp.tile([128, N], f32, name=f"acc{ci}", tag=f"acc{ci}") for ci in range(3)]
            cum_dt = f32 if occlusion else MM_DT
            cumt = [pp.tile([128, 2 * WF], cum_dt, name=f"cum{ci}", tag=f"cum{ci}") for ci in range(3)]

            # ---------------- helpers ----------------
            def fwd(x3, name):
                """x3: 3 tiles [128, N] (layout [H, W]) -> Z: 3 tiles [128, 2*WF]
                (layout [hf, (re|im)])."""
                y1 = [y1p.tile([128, 2 * N], MM_DT, name=f"y1_{m}", tag=f"y1_{m}") for m in range(3)]
                for m in range(3):
                    prr = ps_y1r.tile([128, N], f32, name="y1r", tag="y1r")
                    pii = ps_y1i.tile([128, N], f32, name="y1i", tag="y1i")
                    for k in range(3):
                        nc.tensor.matmul(
                            prr[:], x3[k][:, m * 128:(m + 1) * 128],
                            c1t[k][:, 0:N],
                            start=(k == 0), stop=(k == 2))
                        nc.tensor.matmul(
                            pii[:], x3[k][:, m * 128:(m + 1) * 128],
                            c1t[k][:, N:2 * N],
                            start=(k == 0), stop=(k == 2))
                    nc.any.tensor_copy(y1[m][:, 0:N], prr[:])
                    nc.any.tensor_copy(y1[m][:, N:2 * N], pii[:])
                z = [sp.tile([128, 2 * WF], f32, name=f"z_{name}{m}", tag=f"z_{name}{m}") for m in range(3)]
                for m in range(3):
                    pz = ps_z.tile([128, 2 * WF], f32, name="pz", tag="pz")
                    for k in range(3):
                        nc.tensor.matmul(
                            pz[:], y1[k][:, m * 128:(m + 1) * 128],
                            c2at[k][:],
                            start=(k == 0), stop=False)
                        nc.tensor.matmul(
                            pz[:], y1[k][:, N + m * 128:N + (m + 1) * 128],
                            c2bt[k][:],
                            start=False, stop=(k == 2))
                    nc.any.tensor_copy(z[m][:], pz[:])
                return z

            def inv(f3, name, dst_pool, dst_tag):
                """f3: 3 tiles [128, 2*WF] -> y: 3 tiles [128, N] (layout [H, W])."""
                pch = []
                for mi, (lo, hi) in enumerate(WCH):
                    w = hi - lo
                    t = pbp.tile([w, 2 * N], MM_DT, name=f"p_{mi}", tag=f"p_{mi}")
                    prr = ps_pr.tile([97, N], f32, name="ppr", tag="ppr")
                    pii = ps_pi.tile([97, N], f32, name="ppi", tag="ppi")
                    for k in range(3):
                        # Pr = Zr.T @ cos + Zi.T @ (-sin)
                        nc.tensor.matmul(
                            prr[:w], f3[k][:, lo:hi],
                            c1t[k][:, 0:N],
                            start=(k == 0), stop=False)
                        nc.tensor.matmul(
                            prr[:w], f3[k][:, WF + lo:WF + hi],
                            c1t[k][:, N:2 * N],
                            start=False, stop=(k == 2))
                        # Pi = Zr.T @ sin + Zi.T @ cos
                        nc.tensor.matmul(
                            pii[:w], f3[k][:, lo:hi],
                            sit[k][:],
                            start=(k == 0), stop=False)
                        nc.tensor.matmul(
                            pii[:w], f3[k][:, WF + lo:WF + hi],
                            c1t[k][:, 0:N],
                            start=False, stop=(k == 2))
                    nc.any.tensor_copy(t[:, 0:N], prr[:w])
                    nc.any.tensor_copy(t[:, N:2 * N], pii[:w])
                    pch.append(t)
                y = [dst_pool.tile([128, N], f32, name=f"{dst_tag}{m}", tag=f"{dst_tag}{m}") for m in range(3)]
                for m in range(3):
                    py = ps_y.tile([128, N], f32, name="py", tag="py")
                    for k, (lo, hi) in enumerate(WCH):
                        w = hi - lo
                        nc.tensor.matmul(
                            py[:], pch[k][:w, m * 128:(m + 1) * 128],
                            ibt[0][k][:],
                            start=(k == 0), stop=False)
                        nc.tensor.matmul(
                            py[:], pch[k][:w, N + m * 128:N + (m + 1) * 128],
                            ibt[1][k][:],
                            start=False, stop=(k == 1))
                    nc.any.tensor_copy(y[m][:], py[:])
                return y

            def cplx_mul(z3, pfr, pfi, name):
                """(z3 complex [hf, re|im]) * (pfr + i*pfi) -> 3 tiles [128, 2*WF]."""
                o = [sp.tile([128, 2 * WF], MM_DT, name=f"fm_{name}{ci}", tag=f"fm_{name}{ci}") for ci in range(3)]
                for ci in range(3):
                    zr = z3[ci][:, 0:WF]
                    zi = z3[ci][:, WF:2 * WF]
                    t1 = wp.tile([128, WF], f32, name="cm_t1", tag="cm_t1")
                    t2 = wp.tile([128, WF], f32, name="cm_t2", tag="cm_t2")
                    nc.vector.tensor_mul(t1[:], zr, pfr[ci][:])
                    nc.vector.tensor_mul(t2[:], zi, pfi[ci][:])
                    nc.vector.tensor_sub(o[ci][:, 0:WF], t1[:], t2[:])
                    nc.vector.tensor_mul(t1[:], zr, pfi[ci][:])
                    nc.vector.tensor_mul(t2[:], zi, pfr[ci][:])
                    nc.vector.tensor_add(o[ci][:, WF:2 * WF], t1[:], t2[:])
                return o

            # ---------------- main depth loop (back to front) ----------------
            for dd in range(n_depth - 1, -1, -1):
                first = (dd == n_depth - 1)
                # load Fpsf[d]
                pfr, pfi = [], []
                for ci, (lo, hi) in enumerate(PCH):
                    tr = pfp.tile([128, WF], f32, name=f"pfr{ci}", tag=f"pfr{ci}")
                    ti = pfp.tile([128, WF], f32, name=f"pfi{ci}", tag=f"pfi{ci}")
                    nc.sync.dma_start(tr[:], pf_d[dd, 0, lo:hi, :])
                    nc.sync.dma_start(ti[:], pf_d[dd, 1, lo:hi, :])
                    pfr.append(tr)
                    pfi.append(ti)

                # layered & volume planes
                lay = [wp.tile([128, N], MM_DT, name=f"lay{ci}", tag=f"lay{ci}") for ci in range(3)]
                vol = [wp.tile([128, N], MM_DT, name=f"vol{ci}", tag=f"vol{ci}") for ci in range(3)]
                for ci in range(3):
                    nc.vector.tensor_scalar(
                        lay[ci][:], idxt[ci][:], float(dd), None, op0=Alu.is_equal)
                    nc.vector.tensor_mul(vol[ci][:], lay[ci][:], imgt[ci][:])

                zvol = fwd(vol, "v")
                if occlusion:
                    zlay = fwd(lay, "l")
                    # freq suffix cumsum
                    for ci in range(3):
                        if first:
                            nc.vector.tensor_copy(cumt[ci][:], zlay[ci][:])
                        else:
                            nc.gpsimd.tensor_add(cumt[ci][:], cumt[ci][:], zlay[ci][:])
                    fa = cplx_mul(zlay, pfr, pfi, "a")
                    fv = cplx_mul(zvol, pfr, pfi, "v")
                    alpha = inv(fa, "a", wp, "sa")
                    volb = inv(fv, "v", wp, "sv")
                    if first:
                        # Fcum == Flay at the back plane, so cumb == alpha
                        cumb = alpha
                    else:
                        fc = cplx_mul(cumt, pfr, pfi, "c")
                        cumb = inv(fc, "c", wp, "sc")
                    for ci in range(3):
                        rc = wp.tile([128, N], f32, name="rc", tag="rc")
                        nc.vector.tensor_scalar_add(rc[:], cumb[ci][:], EPS)
                        nc.vector.reciprocal(rc[:], rc[:])
                        bv = wp.tile([128, N], f32, name="bv", tag="bv")
                        nc.vector.tensor_mul(bv[:], volb[ci][:], rc[:])
                        if first:
                            nc.vector.tensor_copy(acct[ci][:], bv[:])
                        else:
                            ba = wp.tile([128, N], f32, name="ba", tag="ba")
                            nc.vector.tensor_mul(ba[:], alpha[ci][:], rc[:])
                            # acc = bv - (ba - 1) * acc
                            t = wp.tile([128, N], f32, name="cmp_t", tag="cmp_t")
                            nc.vector.scalar_tensor_tensor(
                                t[:], ba[:], 1.0, acct[ci][:],
                                op0=Alu.subtract, op1=Alu.mult)
                            nc.vector.tensor_sub(acct[ci][:], bv[:], t[:])
                else:
                    fv = cplx_mul(zvol, pfr, pfi, "v")
                    for ci in range(3):
                        if first:
                            nc.vector.tensor_copy(cumt[ci][:], fv[ci][:])
                        else:
                            nc.vector.tensor_add(cumt[ci][:], cumt[ci][:], fv[ci][:])

            if not occlusion:
                acc2 = inv(cumt, "f", pp, "accf")
                for ci in range(3):
                    nc.vector.tensor_copy(acct[ci][:], acc2[ci][:])

            # store
            for ci, (lo, hi) in enumerate(PCH):
                nc.sync.dma_start(out_d[lo:hi, :], acct[ci][:])

    nc.compile()
    return nc


# =====================================================================
# Host-side PSF pipeline (float64, mirrors reference.py exactly)
# =====================================================================
def _host_psf(heightmap1d, prop_amplitude, prop_phase, H, rho_grid, rho_sampling):
    wl = WAVELENGTHS.reshape(3, 1, 1)
    hm = np.asarray(heightmap1d, np.float64).reshape(1, 1, -1)
    pa = np.asarray(prop_amplitude, np.float64)
    pp_ = np.asarray(prop_phase, np.float64)
    Hm = np.asarray(H, np.float64)
    rg = np.asarray(rho_grid, np.float64)
    rs = np.asarray(rho_sampling, np.float64)

    n_idx = 1.5375 + 0.00829045 / (wl * 1e6) ** 2 - 0.000211046 / (wl * 1e6) ** 4
    phase = 2.0 * np.pi / wl * (n_idx - 1.0) * hm + pp_          # [3,D,M]
    real = np.einsum('wdm,wmr->wdr', pa * np.cos(phase), Hm)
    imag = np.einsum('wdm,wmr->wdr', pa * np.sin(phase), Hm)
    psf1d = (2.0 * np.pi / (wl * SENSOR_DIST)) ** 2 * (real ** 2 + imag ** 2)

    hh = N // 2
    nd = psf1d.shape[1]
    psf_rd = np.empty((3, nd, hh * hh), np.float64)
    for w in range(3):
        sflat = rs[w].reshape(-1)
        for d in range(nd):
            psf_rd[w, d] = np.interp(sflat, rg[w], psf1d[w, d])
    psf_rd = np.maximum(psf_rd, 0.0).astype(np.float32).reshape(3, nd, hh, hh)
    q = np.concatenate([psf_rd[:, :, ::-1, :], psf_rd], axis=-2)
    psf = np.concatenate([q[:, :, :, ::-1], q], axis=-1)          # [3,D,N,N]
    psf = np.fft.fftshift(psf, axes=(-2, -1))
    psf = psf / np.sum(psf, axis=(-2, -1), keepdims=True)
    Fpsf = np.fft.rfft2(psf.astype(np.float64)) / float(N * N)    # [3,D,N,WF]
    pf = np.stack([Fpsf.real, Fpsf.imag], axis=2).astype(np.float32)  # [3,D,2,N,WF]
    return pf


_PROG_CACHE = {}


def kernel(img, depthmap, heightmap1d, prop_amplitude, prop_phase, H,
           rho_grid, rho_sampling, occlusion):
    occ = bool(np.asarray(occlusion).item())
    img = np.asarray(img, np.float32)
    depthmap = np.asarray(depthmap, np.float32)

    pf = _host_psf(heightmap1d, prop_amplitude, prop_phase, H, rho_grid, rho_sampling)

    scale = np.float32(img.max())
    imgs = img / scale                                            # [B,C,N,N] f32
    idxf = np.clip(np.floor(depthmap * np.float32(D)), 0, D - 1)[:, 0]  # [B,N,N]
    c1, si, c2a, c2b, ib = _make_tables()

    if occ not in _PROG_CACHE:
        _PROG_CACHE[occ] = build_program(occ)
    nc = _PROG_CACHE[occ]

    in_maps = []
    for core in range(NCORES):
        b_, c_ = divmod(core, C) if core < B * C else (0, 0)
        in_maps.append({
            "img": np.ascontiguousarray(imgs[b_, c_]),
            "idx": np.ascontiguousarray(idxf[b_]),
            "fpsf": np.ascontiguousarray(pf[c_]),
            "c1": c1, "si": si, "c2a": c2a, "c2b": c2b, "ib": ib,
        })
    t0 = time.perf_counter()
    res_obj = run_bass_kernel_spmd(
        nc, in_maps, list(range(NCORES)),
        trace=bool(os.environ.get("KBASS_TRACE")))
    global LAST
    LAST = {"wall_s": time.perf_counter() - t0,
            "exec_time_ns": res_obj.exec_time_ns,
            "profile_json": res_obj.profile_json}
    res = res_obj.results
    out = np.empty((B, C, N, N), np.float32)
    for core in range(B * C):
        b_, c_ = divmod(core, C)
        out[b_, c_] = res[core]["out"] * scale
    return out



# revision 9
# speedup vs baseline: 1.5268x; 1.5268x over previous
"""Trainium2 Bass kernel for the depth-dependent camera rendering problem.

Strategy
--------
Host (numpy, float64): PSF synthesis (phase -> Hankel einsum -> radial
interp -> quadrant mirror -> fftshift -> normalize), rfft2 of the PSF
(tiny: ~1% of FLOPs), input sharding, and the final 3-fma combine of the
per-core composite partials.

Device (Bass/Tile, 8 NeuronCores, SPMD): core (b, q) owns depth block
d in [4q, 4q+4) of batch b, ALL THREE channels.  Per block, back to
front over local depths:
  lay_d = (idx_local == d);  Flay_d = rfft2(lay_d)      (one per (b,d))
  Fcum_d = Ftail + sum_{d'>=d, local} Flay_d'           (local adds)
     where Ftail = rfft2([idx_local > 3]) -- by DFT linearity this one
     extra forward transform replaces all cross-core communication for
     the suffix-cumsum (it equals the sum of the higher blocks' Flay).
  per channel c: Fvol = rfft2(lay*img_c); three products with the real
  PSF spectrum S[c,d] (fftshift phase ramp folded into the inverse DFT
  tables, so the frequency multiply is REAL); three inverse transforms
  (alpha, volb, cumb); ba = alpha/(cumb+eps), bv = volb/(cumb+eps);
  back-to-front over-op partials  A <- bv + (1-ba)*A,  M <- (ba-1)*M.
Output per core: A[3], M[3] (M = prod(ba-1) = prod(1-ba), block size 4
is even).  Host: out[b,c] = scale * (A_q0 + P_q0*(A_q1 + P_q1*(...))).

DFT as chained matmuls (out = lhsT.T @ rhs contracts the partition axis
and swaps the other two), with real-input Hermitian symmetry:
  fwd pass1 (contract h): y1[w, hf] for hf in [0,193) only   (9 mm)
  fwd pass2 (contract w): full 384 hf rows in a PACKED order where the
    conjugate rows are plain column slices of y1 against a negated
    table -- no mirror copies                                (24 mm)
  packed row -> hf map: [0..127 | 383..256 | 128..192 | 255..193]
  inverse stepA (contract hf): P[wf, h], tables in packed row order
    with the PSF phase ramp folded in                        (24 mm)
  inverse stepB (contract wf): y[h, w] real part with b-weights and
    1/N^2 folded into the tables                             (12 mm)
"""

import os
import time

import numpy as np

import concourse.bass as bass
import concourse.tile as tile
from concourse import bacc, mybir
from concourse.bass_utils import run_bass_kernel_spmd

dt = mybir.dt
Alu = mybir.AluOpType

# ---- problem constants (hardcoded; kernel.py must be self-contained) ----
N = 384            # image H = W
WF = N // 2 + 1    # rfft bins along W = 193
D = 16             # depth planes
DB = 4             # depths per core (block)
NQ = D // DB       # 4 blocks
B, C = 2, 3
EPS = 1e-3
NCORES = 8
WAVELENGTHS = np.array([632e-9, 550e-9, 450e-9])
FOCAL_LENGTH = 50e-3
FOCAL_DEPTH = 1.7
SENSOR_DIST = 1.0 / (1.0 / FOCAL_LENGTH - 1.0 / FOCAL_DEPTH)

MM_DT = dt.float32r   # matmul operand mode (full-rate)

# hf row order is natural (full y1 built via Hermitian mirror copies)
HF_MAP = np.arange(N)

PCH = [(0, 128), (128, 256), (256, 384)]   # partition chunks of 384
WCH = [(0, 128), (128, WF)]                # wf chunks: 128 + 65



# =====================================================================
# Host-side DFT tables
# =====================================================================
def _make_tables():
    k = np.arange(N, dtype=np.float64)
    th = 2.0 * np.pi * np.outer(k, k) / N
    co = np.cos(th)
    sn = np.sin(th)
    c1h = np.concatenate([co[:, :WF], -sn[:, :WF]], axis=1)       # [N, 386]
    c2a = np.concatenate([co[:, :WF], -sn[:, :WF]], axis=1)       # [N, 386]
    c2b = np.concatenate([sn[:, :WF], co[:, :WF]], axis=1)        # [N, 386]
    hfm = HF_MAP.astype(np.float64)
    ang = 2.0 * np.pi * np.outer(hfm, k) / N + (np.pi * hfm / N)[:, None]
    Ac = np.cos(ang)                                              # [384, 384]
    As = np.sin(ang)
    Asn = -As
    b = np.full(WF, 2.0)
    b[0] = 1.0
    b[WF - 1] = 1.0
    wfv = np.arange(WF, dtype=np.float64)
    angW = 2.0 * np.pi * np.outer(wfv, k) / N + (np.pi * wfv / N)[:, None]
    ibr = b[:, None] * np.cos(angW) / float(N * N)                # [193, 384]
    ibin = -b[:, None] * np.sin(angW) / float(N * N)
    f = np.float32
    return (c1h.astype(f), c2a.astype(f), c2b.astype(f),
            Ac.astype(f), As.astype(f), Asn.astype(f),
            ibr.astype(f), ibin.astype(f))


# =====================================================================
# Device program (one core: DB depths x 3 channels)
# =====================================================================
def build_program(occlusion: bool, n_depth: int = DB):
    nc = bacc.Bacc(None, target_bir_lowering=False, debug=False)
    f32 = dt.float32
    occ = bool(occlusion)

    img_d = nc.declare_dram_parameter("img", [C, N, N], f32, isOutput=False)
    idx_d = nc.declare_dram_parameter("idx", [N, N], f32, isOutput=False)
    stab_d = nc.declare_dram_parameter("stab", [C * n_depth, N, WF], f32, isOutput=False)
    c1h_d = nc.declare_dram_parameter("c1h", [N, 2 * WF], f32, isOutput=False)
    c2a_d = nc.declare_dram_parameter("c2a", [N, 2 * WF], f32, isOutput=False)
    c2b_d = nc.declare_dram_parameter("c2b", [N, 2 * WF], f32, isOutput=False)
    ac_d = nc.declare_dram_parameter("tac", [N, N], f32, isOutput=False)
    as_d = nc.declare_dram_parameter("tas", [N, N], f32, isOutput=False)
    asn_d = nc.declare_dram_parameter("tasn", [N, N], f32, isOutput=False)
    ibr_d = nc.declare_dram_parameter("ibr", [WF, N], f32, isOutput=False)
    ibin_d = nc.declare_dram_parameter("ibin", [WF, N], f32, isOutput=False)
    aout_d = nc.declare_dram_parameter("aout", [C, N, N], f32, isOutput=True)
    mout_d = nc.declare_dram_parameter("mout", [C, N, N], f32, isOutput=True)

    with tile.TileContext(nc) as tc:
        with (
            tc.tile_pool(name="const", bufs=1) as cp,
            tc.tile_pool(name="pers", bufs=1) as pp,
            tc.tile_pool(name="work", bufs=2) as wp,
            tc.tile_pool(name="wq", bufs=2) as wq,
            tc.tile_pool(name="flp", bufs=2) as flp,
            tc.tile_pool(name="zs", bufs=1) as zp,
            tc.tile_pool(name="y1p", bufs=2) as y1p,
            tc.tile_pool(name="ppool", bufs=1) as ppl,
            tc.tile_pool(name="psy1", bufs=2, space="PSUM") as ps_y1,
            tc.tile_pool(name="psz", bufs=2, space="PSUM") as ps_z,
            tc.tile_pool(name="psA", bufs=2, space="PSUM") as ps_A,
            tc.tile_pool(name="psy", bufs=2, space="PSUM") as ps_y,
        ):
            # ---- load constants ----
            def load3(dram, cols, tag, dtype=MM_DT, eng=None):
                ts = []
                for ci, (lo, hi) in enumerate(PCH):
                    t = cp.tile([128, cols], dtype, name=f"{tag}{ci}", tag=f"{tag}{ci}")
                    e = eng or (nc.gpsimd if dtype != f32 else nc.sync)
                    e.dma_start(t[:], dram[lo:hi, :])
                    ts.append(t)
                return ts

            c1t = load3(c1h_d, 2 * WF, "c1")
            c2at = load3(c2a_d, 2 * WF, "c2a")
            c2bt = load3(c2b_d, 2 * WF, "c2b")
            act = load3(ac_d, N, "tac")
            ast = load3(as_d, N, "tas")
            asnt = load3(asn_d, N, "tasn")
            ibt = []   # stepB tables [comp][wf-chunk]
            for nm, dram in (("ibr", ibr_d), ("ibin", ibin_d)):
                row = []
                for ci, (lo, hi) in enumerate(WCH):
                    t = cp.tile([hi - lo, N], MM_DT, name=f"{nm}{ci}", tag=f"{nm}{ci}")
                    nc.gpsimd.dma_start(t[:], dram[lo:hi, :])
                    row.append(t)
                ibt.append(row)

            imgt = []
            for c in range(C):
                row = []
                for ci, (lo, hi) in enumerate(PCH):
                    t = cp.tile([128, N], f32, name=f"img{c}{ci}", tag=f"img{c}{ci}")
                    nc.sync.dma_start(t[:], img_d[c, lo:hi, :])
                    row.append(t)
                imgt.append(row)
            idxt = load3(idx_d, N, "idx", dtype=f32, eng=nc.sync)

            stt = []   # S tables [c*n_depth+d][tile]: [128, WF]
            for cd in range(C * n_depth):
                row = []
                for ci, (lo, hi) in enumerate(PCH):
                    t = cp.tile([128, WF], f32, name=f"s{cd}{ci}", tag=f"s{cd}{ci}")
                    nc.scalar.dma_start(t[:], stab_d[cd, lo:hi, :])
                    row.append(t)
                stt.append(row)

            # persistent state
            cumt = [pp.tile([128, 2 * WF], f32, name=f"cum{ci}", tag=f"cum{ci}")
                    for ci in range(3)]
            accA = [[pp.tile([128, N], f32, name=f"accA{c}{ci}", tag=f"accA{c}{ci}")
                     for ci in range(3)] for c in range(C)]
            accM = [[pp.tile([128, N], f32, name=f"accM{c}{ci}", tag=f"accM{c}{ci}")
                     for ci in range(3)] for c in range(C)]
            fsum = None
            if not occ:
                fsum = [[pp.tile([128, 2 * WF], f32, name=f"fs{c}{ci}", tag=f"fs{c}{ci}")
                         for ci in range(3)] for c in range(C)]

            # ---------------- forward DFT ----------------
            def fwd(x3, consume):
                """x3: 3 MM_DT tiles [128, N] ([h, w]).  Calls
                consume(ti, pz) per output tile (PSUM [128, 386],
                natural hf rows, [zr|zi])."""
                y1 = []
                for m in range(3):
                    py1 = ps_y1.tile([128, 2 * WF], f32, name="py1", tag="py1")
                    for k in range(3):
                        nc.tensor.matmul(
                            py1[:], x3[k][:, m * 128:(m + 1) * 128], c1t[k][:],
                            start=(k == 0), stop=(k == 2))
                    # full y1 [y1r(384) | y1i(384)] via Hermitian mirror
                    t = y1p.tile([128, 2 * N], MM_DT, name=f"y1_{m}", tag=f"y1_{m}")
                    nc.any.tensor_copy(t[:, 0:WF], py1[:, 0:WF])
                    nc.any.tensor_copy(t[:, N:N + WF], py1[:, WF:2 * WF])
                    nc.any.tensor_copy(t[:, WF:N], py1[:, WF - 2:0:-1])
                    nc.any.tensor_scalar_mul(t[:, N + WF:2 * N],
                                             py1[:, 2 * WF - 2:WF:-1], -1.0)
                    y1.append(t)
                for ti in range(3):
                    pz = ps_z.tile([128, 2 * WF], f32, name="pz", tag="pz")
                    for k in range(3):
                        nc.tensor.matmul(
                            pz[:], y1[k][:, ti * 128:(ti + 1) * 128], c2at[k][:],
                            start=(k == 0), stop=False)
                        nc.tensor.matmul(
                            pz[:], y1[k][:, N + ti * 128:N + (ti + 1) * 128],
                            c2bt[k][:], start=False, stop=(k == 2))
                    consume(ti, pz)

            # ---------------- inverse stepA ----------------
            def stepA(z3, qname):
                """z3: 3 MM_DT tiles [128, 386] (packed hf) -> P sbuf tiles
                per wf-chunk [wn, 768] ([Pr | Pi])."""
                pouts = []
                for wi, (wlo, whi) in enumerate(WCH):
                    wn = whi - wlo
                    t = ppl.tile([128, 2 * N], MM_DT, name=f"P{qname}{wi}",
                                 tag=f"P{qname}{wi}")
                    for half, (t0, t1) in enumerate(((act, asnt), (ast, act))):
                        pA = ps_A.tile([128, N], f32, name="pA", tag="pA")
                        for k in range(3):
                            nc.tensor.matmul(pA[:wn, :], z3[k][:, wlo:whi],
                                             t0[k][:], start=(k == 0), stop=False)
                            nc.tensor.matmul(pA[:wn, :], z3[k][:, WF + wlo:WF + whi],
                                             t1[k][:], start=False, stop=(k == 2))
                        nc.any.tensor_copy(t[:wn, half * N:(half + 1) * N], pA[:wn, :])
                    pouts.append(t)
                return pouts

            def stepB_tile(pq, ti):
                """One h-tile of the inverse: PSUM [128, N]."""
                py = ps_y.tile([128, N], f32, name="py", tag="py")
                for wi, (wlo, whi) in enumerate(WCH):
                    wn = whi - wlo
                    nc.tensor.matmul(py[:], pq[wi][:wn, ti * 128:(ti + 1) * 128],
                                     ibt[0][wi][:], start=(wi == 0), stop=False)
                    nc.tensor.matmul(py[:], pq[wi][:wn, N + ti * 128:N + (ti + 1) * 128],
                                     ibt[1][wi][:], start=False, stop=(wi == 1))
                return py

            def smul_tiles(src3, s3, tag):
                """zS = z * S into MM_DT sbuf tiles (src may be SBUF)."""
                o = []
                for ci in range(3):
                    t = zp.tile([128, 2 * WF], MM_DT, name=f"{tag}{ci}", tag=f"{tag}{ci}")
                    nc.any.tensor_mul(t[:, 0:WF], src3[ci][:, 0:WF], s3[ci][:])
                    nc.any.tensor_mul(t[:, WF:2 * WF], src3[ci][:, WF:2 * WF], s3[ci][:])
                    o.append(t)
                return o

            # ---------------- tail spectrum -> cum init ----------------
            if occ:
                tail = [wp.tile([128, N], MM_DT, name=f"lay{ci}", tag=f"lay{ci}")
                        for ci in range(3)]
                for ci in range(3):
                    nc.any.tensor_scalar(tail[ci][:], idxt[ci][:],
                                         float(n_depth - 1), None, op0=Alu.is_gt)
                fwd(tail, lambda ti, pz: nc.any.tensor_copy(cumt[ti][:], pz[:]))

            # ---------------- main depth loop (back to front) ----------------
            for dd in range(n_depth - 1, -1, -1):
                first = (dd == n_depth - 1)
                lay = [wp.tile([128, N], MM_DT, name=f"lay{ci}", tag=f"lay{ci}")
                       for ci in range(3)]
                for ci in range(3):
                    nc.any.tensor_scalar(lay[ci][:], idxt[ci][:], float(dd),
                                         None, op0=Alu.is_equal)
                if occ:
                    flay = [flp.tile([128, 2 * WF], f32, name=f"fl{ci}", tag=f"fl{ci}")
                            for ci in range(3)]

                    def eat_lay(ti, pz):
                        nc.any.tensor_copy(flay[ti][:], pz[:])
                        nc.gpsimd.tensor_add(cumt[ti][:], cumt[ti][:], flay[ti][:])
                    fwd(lay, eat_lay)

                for c in range(C):
                    s3 = stt[c * n_depth + dd]
                    vol = [wp.tile([128, N], MM_DT, name=f"vol{ci}", tag=f"vol{ci}")
                           for ci in range(3)]
                    for ci in range(3):
                        nc.any.tensor_mul(vol[ci][:], lay[ci][:], imgt[c][ci][:])
                    zv = [zp.tile([128, 2 * WF], MM_DT, name=f"zv{ci}", tag=f"zv{ci}")
                          for ci in range(3)]

                    def eat_vol(ti, pz):
                        nc.any.tensor_mul(zv[ti][:, 0:WF], pz[:, 0:WF], s3[ti][:])
                        nc.any.tensor_mul(zv[ti][:, WF:2 * WF], pz[:, WF:2 * WF], s3[ti][:])
                    fwd(vol, eat_vol)

                    if not occ:
                        for ci in range(3):
                            if first:
                                nc.any.tensor_copy(fsum[c][ci][:], zv[ci][:])
                            else:
                                nc.gpsimd.tensor_add(fsum[c][ci][:], fsum[c][ci][:],
                                                     zv[ci][:])
                        continue

                    za = smul_tiles(flay, s3, "za")
                    zc = smul_tiles(cumt, s3, "zc")
                    pa = stepA(za, "a")
                    pv = stepA(zv, "v")
                    pc = stepA(zc, "c")
                    for ti in range(3):
                        yc = stepB_tile(pc, ti)
                        rc = wq.tile([128, N], f32, name="rc", tag="rc")
                        nc.any.tensor_scalar_add(rc[:], yc[:], EPS)
                        nc.vector.reciprocal(rc[:], rc[:])
                        ya = stepB_tile(pa, ti)
                        ba = wq.tile([128, N], f32, name="ba", tag="ba")
                        nc.any.tensor_mul(ba[:], ya[:], rc[:])
                        yv = stepB_tile(pv, ti)
                        if first:
                            nc.any.tensor_mul(accA[c][ti][:], yv[:], rc[:])
                            nc.any.tensor_scalar_sub(accM[c][ti][:], ba[:], 1.0)
                        else:
                            bv = wq.tile([128, N], f32, name="bv", tag="bv")
                            nc.any.tensor_mul(bv[:], yv[:], rc[:])
                            t1 = wq.tile([128, N], f32, name="t1", tag="t1")
                            nc.vector.scalar_tensor_tensor(
                                t1[:], ba[:], 1.0, accA[c][ti][:],
                                op0=Alu.subtract, op1=Alu.mult)
                            nc.any.tensor_sub(accA[c][ti][:], bv[:], t1[:])
                            nc.vector.scalar_tensor_tensor(
                                accM[c][ti][:], ba[:], 1.0, accM[c][ti][:],
                                op0=Alu.subtract, op1=Alu.mult)

            if not occ:
                # one inverse of the accumulated spectrum per channel
                for c in range(C):
                    zmm = [zp.tile([128, 2 * WF], MM_DT, name=f"zm{ci}", tag=f"zm{ci}")
                           for ci in range(3)]
                    for ci in range(3):
                        nc.any.tensor_copy(zmm[ci][:], fsum[c][ci][:])
                    pq = stepA(zmm, "f")
                    for ti in range(3):
                        py = stepB_tile(pq, ti)
                        nc.any.tensor_copy(accA[c][ti][:], py[:])
                        nc.vector.memset(accM[c][ti][:], 0.0)

            for c in range(C):
                for ci, (lo, hi) in enumerate(PCH):
                    nc.sync.dma_start(aout_d[c, lo:hi, :], accA[c][ci][:])
                    nc.sync.dma_start(mout_d[c, lo:hi, :], accM[c][ci][:])

    nc.compile()
    return nc


# =====================================================================
# Host-side PSF pipeline (float64, mirrors reference.py exactly)
# =====================================================================
def _host_psf(heightmap1d, prop_amplitude, prop_phase, H, rho_grid, rho_sampling):
    wl = WAVELENGTHS.reshape(3, 1, 1)
    hm = np.asarray(heightmap1d, np.float64).reshape(1, 1, -1)
    pa = np.asarray(prop_amplitude, np.float64)
    pp_ = np.asarray(prop_phase, np.float64)
    Hm = np.asarray(H, np.float64)
    rg = np.asarray(rho_grid, np.float64)
    rs = np.asarray(rho_sampling, np.float64)

    n_idx = 1.5375 + 0.00829045 / (wl * 1e6) ** 2 - 0.000211046 / (wl * 1e6) ** 4
    phase = 2.0 * np.pi / wl * (n_idx - 1.0) * hm + pp_          # [3,D,M]
    real = np.einsum('wdm,wmr->wdr', pa * np.cos(phase), Hm)
    imag = np.einsum('wdm,wmr->wdr', pa * np.sin(phase), Hm)
    psf1d = (2.0 * np.pi / (wl * SENSOR_DIST)) ** 2 * (real ** 2 + imag ** 2)

    hh = N // 2
    nd = psf1d.shape[1]
    psf_rd = np.empty((3, nd, hh * hh), np.float64)
    for w in range(3):
        sflat = rs[w].reshape(-1)
        for d in range(nd):
            psf_rd[w, d] = np.interp(sflat, rg[w], psf1d[w, d])
    psf_rd = np.maximum(psf_rd, 0.0).astype(np.float32).reshape(3, nd, hh, hh)
    q = np.concatenate([psf_rd[:, :, ::-1, :], psf_rd], axis=-2)
    psf = np.concatenate([q[:, :, :, ::-1], q], axis=-1)          # [3,D,N,N]
    psf = np.fft.fftshift(psf, axes=(-2, -1))
    psf = psf / np.sum(psf, axis=(-2, -1), keepdims=True)
    Fpsf = np.fft.rfft2(psf.astype(np.float64))                   # [3,D,N,WF]
    hf = np.arange(N).reshape(-1, 1)
    wf = np.arange(WF).reshape(1, -1)
    S = (Fpsf * np.exp(-1j * np.pi * hf / N) * np.exp(-1j * np.pi * wf / N)).real
    S = S[:, :, HF_MAP, :]                                        # packed rows
    return np.ascontiguousarray(S.astype(np.float32))             # [3,D,384,193]


_PROG_CACHE = {}
_TABLE_CACHE = {}


def kernel(img, depthmap, heightmap1d, prop_amplitude, prop_phase, H,
           rho_grid, rho_sampling, occlusion):
    occ = bool(np.asarray(occlusion).item())
    img = np.asarray(img, np.float32)
    depthmap = np.asarray(depthmap, np.float32)

    S = _host_psf(heightmap1d, prop_amplitude, prop_phase, H, rho_grid, rho_sampling)

    scale = np.float32(img.max())
    imgs = img / scale                                            # [B,C,N,N] f32
    idxf = np.clip(np.floor(depthmap * np.float32(D)), 0, D - 1)[:, 0]  # [B,N,N]
    if "t" not in _TABLE_CACHE:
        _TABLE_CACHE["t"] = _make_tables()
    c1h, c2a, c2b, Ac, As, Asn, ibr, ibin = _TABLE_CACHE["t"]

    if occ not in _PROG_CACHE:
        _PROG_CACHE[occ] = build_program(occ)
    nc = _PROG_CACHE[occ]

    in_maps = []
    for core in range(NCORES):
        b_, q_ = divmod(core, NQ)
        sblk = np.ascontiguousarray(
            S[:, DB * q_:DB * q_ + DB].reshape(C * DB, N, WF))
        in_maps.append({
            "img": np.ascontiguousarray(imgs[b_]),
            "idx": np.ascontiguousarray(idxf[b_] - np.float32(DB * q_)),
            "stab": sblk,
            "c1h": c1h, "c2a": c2a, "c2b": c2b,
            "tac": Ac, "tas": As, "tasn": Asn, "ibr": ibr, "ibin": ibin,
        })
    t0 = time.perf_counter()
    res_obj = run_bass_kernel_spmd(
        nc, in_maps, list(range(NCORES)),
        trace=bool(os.environ.get("KBASS_TRACE")))
    global LAST
    LAST = {"wall_s": time.perf_counter() - t0,
            "exec_time_ns": res_obj.exec_time_ns,
            "profile_json": res_obj.profile_json}
    res = res_obj.results

    out = np.empty((B, C, N, N), np.float32)
    for b_ in range(B):
        # combine blocks front (q=0) to back (q=NQ-1):
        # out = A0 + P0*(A1 + P1*(A2 + P2*A3));  no-occ: P=0 stored, so
        # this reduces to... (no-occ path instead sums the blocks)
        if occ:
            acc = res[b_ * NQ + NQ - 1]["aout"].astype(np.float64)
            for q_ in range(NQ - 2, -1, -1):
                r = res[b_ * NQ + q_]
                acc = r["aout"].astype(np.float64) + r["mout"].astype(np.float64) * acc
        else:
            acc = sum(res[b_ * NQ + q_]["aout"].astype(np.float64)
                      for q_ in range(NQ))
        out[b_] = (scale * acc).astype(np.float32)
    return out
